# revision 47
# baseline (speedup 1.0000x reference)
"""DDiT block kernel for 8 Trainium2 NeuronCores.

Sharding: data-parallel over (batch, sequence-half) -> 8 shards. Each core
processes one batch's full sequence through LN1/K/V (needed for attention),
but only its 512 query tokens through Q/attention/MLP.

Device layout is feature-major (model dim on partitions, tokens on the free
axis), which makes every adaLN modulation a per-partition scalar and lets all
matmuls consume activations without transposes. Host folds the adaLN scale
and LN weight into the qkv/mlp1 weights, and the shift-vectors into biases.
Tokens are rotated per-core so queries are always tokens [0:512).

QKV / attn-out projections AND attention-times-V run in fp8e4m3 DoubleRow
(weights pre-scaled by SW, compensated on readout; probs written fp8 by the
exp with a constant logit shift, softmax-invariant). Scores and the MLP stay
bf16 (fp8 there breaks the 2e-2 gate). LN stats / softmax / residuals fp32.
rstd is computed as exp(-0.5*ln(var+eps)) so the Act engine keeps one
function table (natural_log_exp) resident through the whole attention phase.

Engine split: PE matmuls; Act exp/gelu/readouts; DVE rope + LN normalize +
softmax normalize; Pool (gpsimd) V-readout + denominator broadcast copies.
Weight DMAs ride the Activation HWDGE queue, latency-critical DMAs (x, rope
swaps, attention output, results) ride the SP queue.
"""

import numpy as np
import ml_dtypes

BF = ml_dtypes.bfloat16
F8 = ml_dtypes.float8_e4m3

B, S, D, H, HD = 4, 1024, 1024, 16, 64
Q = 512          # queries per core
KO = 8           # 1024 dim / 128 partitions
MLP = 4096
LN_EPS = 1e-5
SW = 64.0        # fp8 weight pre-scale (compensated on readout)
SHIFT = 2.5      # exp logit shift (softmax-invariant, keeps fp8 probs finite)

_CACHE = {}


# ----------------------------------------------------------------------------
# host-side layout helpers
# ----------------------------------------------------------------------------

def _pieces(W, m_piece, dt=BF, scale=1.0):
    """[K, M] weight -> [n_pieces, 128, K//128, m_piece], contiguous."""
    K, M = W.shape
    ko = K // 128
    Wr = (np.asarray(W, np.float32) * scale).reshape(ko, 128, M).transpose(1, 0, 2)
    n = M // m_piece
    out = Wr.reshape(128, ko, n, m_piece).transpose(2, 0, 1, 3)
    return np.ascontiguousarray(out.astype(dt))


def _pvec(v):
    """[M] vector -> [128, M//128] f32 (partition-major chunks)."""
    v = np.asarray(v, np.float32)
    return np.ascontiguousarray(v.reshape(-1, 128).T)


# ----------------------------------------------------------------------------
# device program
# ----------------------------------------------------------------------------

def _build_program(repeat=1):
    import concourse.bass as bass
    import concourse.mybir as mybir
    import concourse.tile as tile
    from concourse import bacc

    f32 = mybir.dt.float32
    bf = mybir.dt.bfloat16
    f8 = mybir.dt.float8e4
    AF = mybir.ActivationFunctionType
    ALU = mybir.AluOpType
    DR = mybir.MatmulPerfMode.DoubleRow
    ts = bass.ts

    nc = bacc.Bacc("TRN2", target_bir_lowering=False, debug=False,
                   enable_asserts=False)

    def din(name, shape, dt=bf):
        return nc.dram_tensor(name, shape, dt, kind="ExternalInput").ap()

    xb_d = din("xb", [D, S])                      # bf16 x, feature-major
    wq_d = din("wq", [2, 128, KO, 512], f8)
    wk_d = din("wk", [2, 128, KO, 512], f8)
    wv_d = din("wv", [2, 128, KO, 512], f8)
    wo_d = din("wao", [2, 128, KO, 512], f8)
    w1_d = din("wm1", [8, 128, KO, 512])
    w2_d = din("wm2", [8, 128, 32, 128])
    cc_d = din("cc", [128, 1536])                 # cos: [q 512 | k 1024]
    ss_d = din("ss", [128, 1536])                 # +-sin, signs baked per row
    bv_d = din("bvec", [128, 80], f32)
    yt_d = nc.dram_tensor("yt", [D, Q], bf, kind="ExternalOutput").ap()

    with tile.TileContext(nc) as tc:
        with tc.tile_pool(name="sb", bufs=1) as sb, \
             tc.tile_pool(name="ps", bufs=1, space="PSUM") as ps:
            for _rep in range(repeat):
                # Pin the natural_log_exp table (ln+exp+identity+copy) up
                # front so the Ln/Exp rstd trick and the attention exps all
                # share one residency; only the gelu set loads later.
                nc.scalar.add_instruction(mybir.InstLoadActFuncSet(
                    act_func_set_id=6,
                    name=nc.get_next_instruction_name(),
                    engine=mybir.EngineType.Activation, ins=[], outs=[]))

                def psum():
                    return ps.tile([128, 512], f32, tag="p", bufs=2, name="pt")

                def psumv():
                    return ps.tile([128, 512], f32, tag="pv", bufs=2, name="pvt")

                def psum2():
                    return ps.tile([128, 1024], f32, tag="p2", bufs=2, name="pt2")

                def tmpf():
                    return sb.tile([128, 512], f32, tag="tmpf", bufs=2, name="tf")

                # ---- P0: DMA schedule. x token-half tb0 first, split across
                # the SP and Act HWDGE queues; weights stream on the Act
                # queue behind the critical pieces. ----
                xb_r = xb_d.rearrange("(ko p) t -> p ko t", p=128)
                xb8 = [sb.tile([128, S], bf, tag="xb8", bufs=8, name="xb")
                       for _ in range(KO)]
                for ko in range(KO):
                    eng = nc.sync if ko % 2 == 0 else nc.scalar
                    eng.dma_start(xb8[ko][:], xb_r[:, ko, :])

                def wpiece(dram, i, shape, tag, bufs, dt=bf, eng=None):
                    eng = eng or nc.scalar
                    t = sb.tile(shape, dt, tag=tag, bufs=bufs, name=tag)
                    eng.dma_start(t[:], dram[i])
                    return t

                wq_sb = [None, None]
                wk_sb = [None, None]
                wv_sb = [None, None]
                wq_sb[0] = wpiece(wq_d, 0, [128, KO, 512], "wq", 2, dt=f8)
                wk_sb[0] = wpiece(wk_d, 0, [128, KO, 512], "wk", 2, dt=f8)
                wv_sb[0] = wpiece(wv_d, 0, [128, KO, 512], "wv", 2, dt=f8)
                csb = sb.tile([128, 1536], bf, tag="cs", bufs=2)
                ssb = sb.tile([128, 1536], bf, tag="cs", bufs=2)
                nc.sync.dma_start(csb[:], cc_d[:])
                nc.sync.dma_start(ssb[:], ss_d[:])
                bvec = sb.tile([128, 80], f32, tag="bias", bufs=1, name="bvec")
                nc.sync.dma_start(bvec[:], bv_d[:])
                bq_s, bk_s, bo_s, gm_s = (bvec[:, 8 * i:8 * i + 8]
                                          for i in range(4))
                b1_s = bvec[:, 32:64]
                b2_s, gp_s = bvec[:, 64:72], bvec[:, 72:80]
                # remaining weight pieces are staggered one-per-iteration
                # through the pair loop (SP queue; big transfers must not
                # head-of-line-block the rope swap DMAs)
                wo_sb = []
                w1_sb = []
                w2_sb = []

                ones_b = sb.tile([128, 128], bf, tag="ones", bufs=1)
                nc.vector.memset(ones_b[:], 1.0)
                eps_ap = sb.tile([128, 1], f32, tag="eps", bufs=1)
                nc.vector.memset(eps_ap[:], LN_EPS)
                shift_ap = sb.tile([128, 1], f32, tag="shift", bufs=1)
                nc.vector.memset(shift_ap[:], -SHIFT)

                # ---- P1: LN1, pipelined by token half. rstd computed as
                # exp(-0.5*ln(var+eps)) so only the natural_log_exp Act table
                # is needed until the MLP. ----
                s1t = psum2()
                s2t = psum2()
                ps_s1 = [s1t[:, 0:512], s1t[:, 512:1024]]
                ps_s2 = [s2t[:, 0:512], s2t[:, 512:1024]]
                mu01 = sb.tile([128, 1024], bf, tag="stats16", bufs=3, name="mu01")
                rstd01 = sb.tile([128, 1024], bf, tag="stats16", bufs=3, name="rstd01")
                g_all = sb.tile([128, KO, S], f8, tag="gall", bufs=1, name="gall")

                def ln1_stats(tb):
                    for ko in range(KO):
                        sqk = sb.tile([128, Q], bf, tag="qslab", bufs=5, name="sqk")
                        nc.vector.tensor_tensor(sqk[:], xb8[ko][:, ts(tb, 512)],
                                                xb8[ko][:, ts(tb, 512)], ALU.mult)
                        nc.tensor.matmul(ps_s1[tb], ones_b[:],
                                         xb8[ko][:, ts(tb, 512)],
                                         start=(ko == 0), stop=(ko == KO - 1))
                        nc.tensor.matmul(ps_s2[tb], ones_b[:], sqk[:],
                                         start=(ko == 0), stop=(ko == KO - 1))
                    with nc.allow_low_precision(reason="bf16 LN mean"):
                        nc.vector.tensor_scalar_mul(mu01[:, ts(tb, 512)],
                                                    ps_s1[tb], 1.0 / D)
                    ex2 = tmpf()
                    nc.vector.tensor_scalar_mul(ex2[:], ps_s2[tb], 1.0 / D)
                    var = tmpf()
                    nc.vector.tensor_tensor(var[:], mu01[:, ts(tb, 512)],
                                            mu01[:, ts(tb, 512)], ALU.mult)
                    nc.vector.tensor_tensor(var[:], ex2[:], var[:], ALU.subtract)
                    lnv = tmpf()
                    nc.scalar.activation(lnv[:], var[:], AF.Ln, bias=eps_ap[:])
                    with nc.allow_low_precision(reason="bf16 LN rstd"):
                        nc.scalar.activation(rstd01[:, ts(tb, 512)], lnv[:],
                                             AF.Exp, scale=-0.5)

                def ln1_norm(tb):
                    # DVE/Pool split, writing f8 directly (no Act hop: the
                    # cast would add per-slab latency to the g_all chain and
                    # crowd the Act queue ahead of the first exps).
                    for ko in range(KO):
                        eng = nc.vector if ko < 6 else nc.gpsimd
                        tm = sb.tile([128, Q], bf, tag="qslab", bufs=5, name="tm")
                        eng.tensor_tensor(tm[:], xb8[ko][:, ts(tb, 512)],
                                          mu01[:, ts(tb, 512)], ALU.subtract)
                        with nc.allow_low_precision(reason="fp8 activations"):
                            eng.tensor_tensor(g_all[:, ko, ts(tb, 512)],
                                              tm[:], rstd01[:, ts(tb, 512)],
                                              ALU.mult)

                ln1_stats(0)
                ln1_stats(1)
                ln1_norm(0)

                # ---- P2: V projection (fp8 DR), token-major, + ones column
                # at 64 for the softmax denominator ----
                v_sb = sb.tile([128, KO, H, 66], f8, tag="m16v", bufs=1, name="vsb")
                nc.vector.memset(v_sb[:, :, :, 64:66], 1.0)

                def vproj(nb, tos, force_dve=False):
                    # readout alternates Act/DVE (Pool cannot read PSUM)
                    for to in tos:
                        pv = psum()
                        for kp in range(KO // 2):
                            nc.tensor.matmul(pv[:],
                                             g_all[:, 2 * kp:2 * kp + 2, ts(to, 128)],
                                             wv_sb[nb][:, 2 * kp:2 * kp + 2, :],
                                             start=(kp == 0), stop=(kp == KO // 2 - 1),
                                             perf_mode=DR)
                        dst = v_sb[:, to, nb * 8:(nb + 1) * 8, 0:64]
                        src = pv[:].rearrange("p (h d) -> p h d", d=64)
                        with nc.allow_low_precision(reason="fp8 v"):
                            if to % 2 == 0 and not force_dve:
                                nc.scalar.activation(dst, src, AF.Identity,
                                                     scale=1.0 / SW)
                            else:
                                nc.vector.tensor_scalar_mul(dst, src, 1.0 / SW)

                def qkproj(jo, tbs=(0, 1), qk=None):
                    """PE: fp8-DR q/k matmuls; DVE readout into one [128,1536]
                    qk tile (q tokens 0:512 | k tokens 0:1024)."""
                    if qk is None:
                        qk = sb.tile([128, 1536], bf, tag="qk", bufs=2, name="qk")
                    if 0 in tbs:
                        pq = psum()
                        for kp in range(KO // 2):
                            nc.tensor.matmul(pq[:],
                                             wq_sb[jo // 4][:, 2 * kp:2 * kp + 2, ts(jo % 4, 128)],
                                             g_all[:, 2 * kp:2 * kp + 2, 0:Q],
                                             start=(kp == 0), stop=(kp == KO // 2 - 1),
                                             perf_mode=DR)
                        nc.vector.tensor_scalar(qk[:, 0:512], pq[:], 1.0 / SW,
                                                bq_s[:, jo:jo + 1], ALU.mult, ALU.add)
                    for tb in tbs:
                        pk = psum()
                        for kp in range(KO // 2):
                            nc.tensor.matmul(pk[:],
                                             wk_sb[jo // 4][:, 2 * kp:2 * kp + 2, ts(jo % 4, 128)],
                                             g_all[:, 2 * kp:2 * kp + 2, ts(tb, 512)],
                                             start=(kp == 0), stop=(kp == KO // 2 - 1),
                                             perf_mode=DR)
                        nc.vector.tensor_scalar(qk[:, 512 + tb * 512:1024 + tb * 512],
                                                pk[:], 1.0 / SW,
                                                bk_s[:, jo:jo + 1], ALU.mult, ALU.add)
                    return qk

                def swap_start(qk, qsw=None, lo=0, hi=1536):
                    """swap-halves via SP DMA into a fresh tile."""
                    if qsw is None:
                        qsw = sb.tile([128, 1536], bf, tag="qsw", bufs=2, name="qsw")
                    for gI in range(2):
                        r = gI * 64
                        nc.sync.dma_start(qsw[r:r + 32, lo:hi],
                                          qk[r + 32:r + 64, lo:hi])
                        nc.sync.dma_start(qsw[r + 32:r + 64, lo:hi],
                                          qk[r:r + 32, lo:hi])
                    return qsw

                def rope_finish(qk, qsw, qkr=None, t1=None, lo=0, hi=1536):
                    # qsw*sin on Pool (SBUF-only, frees DVE); t1 and the add
                    # stay DVE
                    if t1 is None:
                        t1 = sb.tile([128, 1536], bf, tag="qsw", bufs=2, name="qt1")
                    nc.vector.tensor_tensor(t1[:, lo:hi], qk[:, lo:hi],
                                            csb[:, lo:hi], ALU.mult)
                    nc.gpsimd.tensor_tensor(qsw[:, lo:hi], qsw[:, lo:hi],
                                            ssb[:, lo:hi], ALU.mult)
                    if qkr is None:
                        qkr = sb.tile([128, 1536], bf, tag="qkr", bufs=2, name="qkr")
                    nc.vector.tensor_tensor(qkr[:, lo:hi], t1[:, lo:hi],
                                            qsw[:, lo:hi], ALU.add)
                    return qkr

                # ---- P3: software-pipelined head-pair loop, two pairs deep.
                # Act runs exp back-to-back (the period-setter); pair h+2's
                # projections and pair h+1's rope run inside iteration h so
                # the next pair's scores never wait; attnV (fp8 DR, cheap)
                # trails by two halves. ----
                # prologue: pair 0 runs from token-half 0 alone (rope split
                # at the tb boundary) while tb1's LN/proj chain catches up.
                with tc.high_priority():
                    qk0 = qkproj(0, tbs=(0,))
                    qsw0 = swap_start(qk0, lo=0, hi=1024)
                    t1_0 = sb.tile([128, 1536], bf, tag="qsw", bufs=2,
                                   name="qt1")
                    qkr_cur = rope_finish(qk0, qsw0, t1=t1_0, lo=0, hi=1024)
                wv_sb[1] = wpiece(wv_d, 1, [128, KO, 512], "wv", 2, dt=f8,
                                  eng=nc.sync)
                vproj(0, range(4))

                probs = {}       # (hp, half, sub) -> [128,1024] f8 tile
                po2_all = {}     # hp -> [po2_sub0, po2_sub1]
                qkr_next = None

                def scores_half(hp, qkr, half):
                    pbig = {}
                    for sub in range(2):
                        r0 = sub * 64
                        big = psum2()
                        for kk in range(2):
                            kt = half * 2 + kk
                            nc.tensor.matmul(big[:, ts(kk, 512)],
                                             qkr[r0:r0 + 64,
                                                 512 + kt * 128:640 + kt * 128],
                                             qkr[r0:r0 + 64, 0:Q],
                                             start=True, stop=True,
                                             tile_position=(r0, 0))
                        pbig[sub] = big
                    for sub in range(2):
                        pb = sb.tile([128, 1024], f8, tag="probs", bufs=6, name="pb")
                        with nc.allow_low_precision(reason="fp8 probs"):
                            nc.scalar.activation(pb[:], pbig[sub][:], AF.Exp,
                                                 bias=shift_ap[:], scale=0.125)
                        probs[(hp, half, sub)] = pb

                def attnv_half(hp, half):
                    if half == 0:
                        po2_all[hp] = [psumv(), psumv()]
                    po2 = po2_all[hp]
                    for sub in range(2):
                        h = 2 * hp + sub
                        pb = probs.pop((hp, half, sub))
                        nc.tensor.matmul(po2[sub][0:65, :],
                                         v_sb[:, 2 * half:2 * half + 2, h, 0:65],
                                         pb[:].rearrange("p (two q) -> p two q", two=2),
                                         start=(half == 0), stop=(half == 3),
                                         perf_mode=DR)

                rcp_all = {}

                def den_rcp(hp):
                    """reciprocal of both heads' denominators (DVE, early)."""
                    rcps = []
                    for sub in range(2):
                        rcp = sb.tile([65, 512], bf, tag="rcp", bufs=2, name="rcp")
                        with nc.allow_low_precision(reason="bf16 softmax denom"):
                            nc.vector.reciprocal(rcp[64:65, :],
                                                 po2_all[hp][sub][64:65, :])
                        rcps.append(rcp)
                    rcp_all[hp] = rcps

                def den_fin(hp):
                    """broadcast (PE) + normalized fp8 output; two SP DMAs
                    into oT. The last pair's rb copies ride Act (idle in the
                    drain window) to shorten the P4 critical chain."""
                    po2 = po2_all.pop(hp)
                    rcps = rcp_all.pop(hp)
                    o2b = sb.tile([64, 1024], f8, tag="o2b", bufs=3, name="o2b")
                    prbs = []
                    for sub in range(2):
                        prb = psum()
                        nc.tensor.matmul(prb[0:64, :], ones_b[64:65, 0:64],
                                         rcps[sub][64:65, :], start=True, stop=True)
                        prbs.append(prb)
                    for sub in range(2):
                        rb = sb.tile([64, 512], bf, tag="rb", bufs=2, name="rb")
                        if hp == 7:
                            nc.scalar.copy(rb[:], prbs[sub][0:64, :])
                        else:
                            nc.vector.tensor_copy(rb[:], prbs[sub][0:64, :])
                        with nc.allow_low_precision(reason="fp8 attn output"):
                            nc.vector.tensor_tensor(o2b[:, ts(sub, 512)],
                                                    po2[sub][0:64, :], rb[:],
                                                    ALU.mult)
                    nc.sync.dma_start(oT_all[0:64, hp, :], o2b[:, 0:512])
                    nc.sync.dma_start(oT_all[64:128, hp, :], o2b[:, 512:1024])

                oT_all = sb.tile([128, KO, Q], f8, tag="oTall", bufs=1, name="oTall")

                qkrs = {0: qkr_cur}
                with tc.high_priority():
                    scores_half(0, qkrs[0], 0)
                    scores_half(0, qkrs[0], 1)
                ln1_norm(1)
                qkproj(0, tbs=(1,), qk=qk0)
                swap_start(qk0, qsw=qsw0, lo=1024, hi=1536)
                wq_sb[1] = wpiece(wq_d, 1, [128, KO, 512], "wq", 2, dt=f8,
                                  eng=nc.sync)
                rope_finish(qk0, qsw0, qkr=qkr_cur, t1=t1_0, lo=1024, hi=1536)
                qks = {1: qkproj(1)}

                for hp in range(8):
                    qkr = qkrs.pop(hp)
                    if hp + 1 <= 7:
                        qsw_next = swap_start(qks[hp + 1])
                    # staggered weight prefetch: one piece per iteration
                    # so transfers never back up ahead of the swap DMAs
                    if hp == 0:
                        wk_sb[1] = wpiece(wk_d, 1, [128, KO, 512], "wk", 2,
                                          dt=f8, eng=nc.sync)
                    elif hp == 1:
                        wo_sb.append(wpiece(wo_d, 0, [128, KO, 512], "wo", 2,
                                            dt=f8, eng=nc.sync))
                        w1_sb.append(wpiece(w1_d, 0, [128, KO, 512], "w1", 3,
                                            eng=nc.sync))
                    elif hp == 2:
                        wo_sb.append(wpiece(wo_d, 1, [128, KO, 512], "wo", 2,
                                            dt=f8, eng=nc.sync))
                        w1_sb.append(wpiece(w1_d, 1, [128, KO, 512], "w1", 3,
                                            eng=nc.sync))
                    elif hp == 3:
                        w1_sb.append(wpiece(w1_d, 2, [128, KO, 512], "w1", 3,
                                            eng=nc.sync))
                    elif hp == 4:
                        w2_sb.append(wpiece(w2_d, 0, [128, 32, 128], "w2", 2,
                                            eng=nc.sync))
                    elif hp == 5:
                        w2_sb.append(wpiece(w2_d, 1, [128, 32, 128], "w2", 2,
                                            eng=nc.sync))
                    if hp > 0:
                        attnv_half(hp - 1, 2)
                        attnv_half(hp - 1, 3)
                        den_rcp(hp - 1)
                    if hp + 1 <= 7:
                        qkrs[hp + 1] = rope_finish(qks.pop(hp + 1), qsw_next)
                    if hp > 0:
                        scores_half(hp, qkr, 1)
                    if hp + 2 <= 7:
                        qks[hp + 2] = qkproj(hp + 2)
                    if hp > 0:
                        den_fin(hp - 1)
                    scores_half(hp, qkr, 2)
                    if hp == 0:
                        vproj(0, range(4, 8), force_dve=True)
                    if 1 <= hp <= 4:
                        vproj(1, (2 * (hp - 1), 2 * (hp - 1) + 1))
                    scores_half(hp, qkr, 3)
                    attnv_half(hp, 0)
                    attnv_half(hp, 1)
                    if hp + 1 <= 7:
                        scores_half(hp + 1, qkrs[hp + 1], 0)
                attnv_half(7, 2)
                attnv_half(7, 3)
                den_rcp(7)

                # ---- P4: attn out (fp8 DR) + gated residual (bf16), fused
                # with LN2 stats. Wave A (do 0-3) partials over pairs 0-5
                # run while the last exps drain (their psum slots free as
                # exp(7,3) completes); only the kp=3 step waits on pair 7's
                # output. ----
                den_fin(7)

                x2 = []
                p1 = psumv()
                p2 = psumv()
                for do in range(KO):
                    py = psum()
                    for kp in range(KO // 2):
                        nc.tensor.matmul(py[:],
                                         wo_sb[do // 4][:, 2 * kp:2 * kp + 2, ts(do % 4, 128)],
                                         oT_all[:, 2 * kp:2 * kp + 2, :],
                                         start=(kp == 0), stop=(kp == KO // 2 - 1),
                                         perf_mode=DR)
                    t = sb.tile([128, Q], bf, tag="qslab", bufs=5, name="tao")
                    nc.scalar.activation(t[:], py[:], AF.Identity,
                                         bias=bo_s[:, do:do + 1],
                                         scale=gm_s[:, do:do + 1])
                    xx = sb.tile([128, Q], bf, tag="act2k", bufs=9, name="x2")
                    nc.vector.tensor_tensor(xx[:], t[:], xb8[do][:, 0:Q], ALU.add)
                    x2.append(xx)
                    sq2 = sb.tile([128, Q], bf, tag="qslab", bufs=5, name="sq2")
                    nc.vector.tensor_tensor(sq2[:], xx[:], xx[:], ALU.mult)
                    nc.tensor.matmul(p1[:], ones_b[:], xx[:], start=(do == 0),
                                     stop=(do == KO - 1))
                    nc.tensor.matmul(p2[:], ones_b[:], sq2[:], start=(do == 0),
                                     stop=(do == KO - 1))

                # late MLP weight pieces stream on SP during MLP1
                w1_sb += [wpiece(w1_d, i, [128, KO, 512], "w1", 3, eng=nc.sync)
                          for i in range(3, 8)]
                w2_sb += [wpiece(w2_d, i, [128, 32, 128], "w2", 2, eng=nc.sync)
                          for i in range(2, 8)]

                # ---- P5: LN2 tail (same ln/exp rstd trick) ----
                mu16 = sb.tile([128, 512], bf, tag="stats16", bufs=3, name="mu16b")
                with nc.allow_low_precision(reason="bf16 LN mean"):
                    nc.vector.tensor_scalar_mul(mu16[:], p1[:], 1.0 / D)
                ex2 = tmpf()
                nc.vector.tensor_scalar_mul(ex2[:], p2[:], 1.0 / D)
                var = tmpf()
                nc.vector.tensor_tensor(var[:], mu16[:], mu16[:], ALU.mult)
                nc.vector.tensor_tensor(var[:], ex2[:], var[:], ALU.subtract)
                lnv = tmpf()
                nc.scalar.activation(lnv[:], var[:], AF.Ln, bias=eps_ap[:])
                rstd16 = sb.tile([128, 512], bf, tag="stats16", bufs=3, name="rstd16b")
                with nc.allow_low_precision(reason="bf16 LN rstd"):
                    nc.scalar.activation(rstd16[:], lnv[:], AF.Exp, scale=-0.5)
                g2 = []
                for ko in range(KO):
                    tm2 = sb.tile([128, 512], bf, tag="qslab", bufs=5, name="tm2")
                    nc.vector.tensor_tensor(tm2[:], x2[ko][:], mu16[:], ALU.subtract)
                    gk = sb.tile([128, Q], bf, tag="g2", bufs=8, name="g2")
                    nc.vector.tensor_tensor(gk[:], tm2[:], rstd16[:], ALU.mult)
                    g2.append(gk)

                # ---- P6/P7: MLP (bf16) ----
                m16 = sb.tile([128, 32, Q], bf, tag="m16v", bufs=1, name="m16")
                for mo in range(32):
                    pm = psum()
                    for ko in range(KO):
                        nc.tensor.matmul(pm[:], w1_sb[mo // 4][:, ko, ts(mo % 4, 128)],
                                         g2[ko][:], start=(ko == 0),
                                         stop=(ko == KO - 1))
                    nc.scalar.activation(m16[:, mo], pm[:], AF.Gelu_apprx_tanh,
                                         bias=b1_s[:, mo:mo + 1], scale=1.0)
                yt_r = yt_d.rearrange("(ko p) t -> p ko t", p=128)
                for do in range(KO):
                    chunks = [(0, 512)] if do < 7 else [(0, 256), (256, 512)]
                    for lo, hi in chunks:
                        pz = psum()
                        for ko in range(32):
                            nc.tensor.matmul(pz[:, 0:hi - lo], w2_sb[do][:, ko, :],
                                             m16[:, ko, lo:hi], start=(ko == 0),
                                             stop=(ko == 31))
                        t = tmpf()
                        nc.scalar.activation(t[:, 0:hi - lo], pz[:, 0:hi - lo],
                                             AF.Identity,
                                             bias=b2_s[:, do:do + 1],
                                             scale=gp_s[:, do:do + 1])
                        yk = sb.tile([128, Q], bf, tag="yout", bufs=2, name="yout")
                        with nc.allow_low_precision(reason="bf16 output"):
                            nc.vector.tensor_tensor(yk[:, 0:hi - lo],
                                                    t[:, 0:hi - lo],
                                                    x2[do][:, lo:hi], ALU.add)
                        nc.scalar.dma_start(yt_r[:, do, lo:hi], yk[:, 0:hi - lo])

    nc.compile()
    return nc


# ----------------------------------------------------------------------------
# host wrapper
# ----------------------------------------------------------------------------

def _prep_shared(inputs):
    x = np.asarray(inputs["x"], np.float32)
    c = np.asarray(inputs["c"], np.float32)
    w_ada = np.asarray(inputs["w_ada"], np.float32)
    b_ada = np.asarray(inputs["b_ada"], np.float32)
    w_qkv = np.asarray(inputs["w_qkv"], np.float32)
    w_ao = np.asarray(inputs["w_attn_out"], np.float32)
    w_m1 = np.asarray(inputs["w_mlp1"], np.float32)
    w_m2 = np.asarray(inputs["w_mlp2"], np.float32)

    mod = c @ w_ada + b_ada
    sh_msa, sc_msa, g_msa, sh_mlp, sc_mlp, g_mlp = np.split(mod, 6, axis=1)
    ln1 = np.asarray(inputs["w_ln1"], np.float32) * (1.0 + sc_msa)   # [4, D]
    ln2 = np.asarray(inputs["w_ln2"], np.float32) * (1.0 + sc_mlp)

    shared = {}
    for b in range(B):
        Wq = w_qkv[:, :D] * ln1[b][:, None]
        Wk = w_qkv[:, D:2 * D] * ln1[b][:, None]
        Wv = w_qkv[:, 2 * D:] * ln1[b][:, None]
        bqkv = sh_msa[b] @ w_qkv
        W1 = w_m1 * ln2[b][:, None]
        bm1 = sh_mlp[b] @ w_m1 + np.asarray(inputs["b_mlp1"], np.float32)
        bvec = np.concatenate([
            _pvec(bqkv[:D]), _pvec(bqkv[D:2 * D]),
            _pvec((bqkv[2 * D:] @ w_ao) * g_msa[b]),
            _pvec(g_msa[b] / SW), _pvec(bm1),
            _pvec(np.asarray(inputs["b_mlp2"], np.float32) * g_mlp[b]),
            _pvec(g_mlp[b]),
        ], axis=1)
        shared[b] = dict(
            wq=_pieces(Wq, 512, F8, SW), wk=_pieces(Wk, 512, F8, SW),
            wv=_pieces(Wv, 512, F8, SW),
            wm1=_pieces(W1, 512),
            bvec=np.ascontiguousarray(bvec),
        )
    wao_p = _pieces(w_ao, 512, F8, SW)
    wm2_p = _pieces(w_m2, 128)
    cos = np.asarray(inputs["cos"], np.float32)
    sin = np.asarray(inputs["sin"], np.float32)
    return shared, wao_p, wm2_p, x, cos, sin


def _make_in_maps(inputs):
    shared, wao_p, wm2_p, x, cos, sin = _prep_shared(inputs)
    in_maps = []
    for core in range(8):
        b, half = core // 2, core % 2
        qlo = half * Q
        order = np.concatenate([np.arange(qlo, qlo + Q), np.arange(0, qlo),
                                np.arange(qlo + Q, S)])
        xT = x[b][order].T
        cosT = cos[order].T                      # [32, S]
        sinT = sin[order].T
        cc = np.concatenate([cosT] * 4, 0)
        ss = np.concatenate([-sinT, sinT, -sinT, sinT], 0)
        cc2 = np.concatenate([cc[:, 0:512], cc], 1).astype(BF)
        ss2 = np.concatenate([ss[:, 0:512], ss], 1).astype(BF)
        sh = shared[b]
        in_maps.append({
            "xb": np.ascontiguousarray(xT.astype(BF)),
            "wq": sh["wq"], "wk": sh["wk"], "wv": sh["wv"],
            "wao": wao_p, "wm1": sh["wm1"], "wm2": wm2_p,
            "cc": np.ascontiguousarray(cc2), "ss": np.ascontiguousarray(ss2),
            "bvec": sh["bvec"],
        })
    return in_maps


def kernel(**inputs):
    from concourse import bass_utils

    if "nc" not in _CACHE:
        _CACHE["nc"] = _build_program()
    nc = _CACHE["nc"]

    in_maps = _make_in_maps(inputs)
    res = bass_utils.run_bass_kernel_spmd(nc, in_maps, core_ids=list(range(8)))

    y = np.zeros((B, S, D), np.float32)
    for core in range(8):
        b, half = core // 2, core % 2
        qlo = half * Q
        y[b, qlo:qlo + Q] = np.asarray(res.results[core]["yt"],
                                       np.float32).T
    return y
